# revision 4
# baseline (speedup 1.0000x reference)
"""Trainium2 Bass kernel v3 for AvgReadout segment mean + L2 normalize.

Architecture (per core; src-range sharding, no collectives):
- Edges sorted by (superblock of 5 seg-blocks, dst-quarter) into packed
  runs; one dma_gather call per run (<=23 subtiles, 4 SWDGE queues).
- Pads (up to the cross-core run capacity) gather emb row 0 and carry an
  out-of-range srcloc sentinel, so their one-hot column is all zero; no
  registers / -1 index trimming (the 2.8% extra descriptors are cheaper
  than the serialization the register WAR chain caused).
- One-hot built per (block, quarter) span by a single batched DVE
  is_equal against a per-block shifted iota table; subtiles may mix
  edges of adjacent blocks (absolute src-in-superblock compare).
- Per block: PE matmul chain over its span subtiles accumulates
  vsum in PSUM; epilogue computes 1/max(||v||,eps) and writes fp16.
- Output DMA batched per superblock.
"""

import numpy as np
from contextlib import ExitStack

N_SPOT = 100000
D = 128
P = 128
NCORES = 8
SEG_PER_CORE = 12500
NBLK = (SEG_PER_CORE + P - 1) // P  # 98
NQ = 4
QROWS = N_SPOT // NQ  # 25000
SB = 5                # blocks per superblock
NSB = (NBLK + SB - 1) // SB  # 20
NRUNS = NSB * NQ      # 80
CALL_CAP = 23         # subtiles per dma_gather call (< 3072 idx ring)
NQUEUES = 4
DMA_SCRATCH = 49152
SENT = 999.0


def preprocess(emb, mask):
    emb = np.asarray(emb, dtype=np.float32)
    emb16 = np.ascontiguousarray(emb.astype(np.float16))
    mask = np.asarray(mask)
    src = mask[0].astype(np.int64, copy=False)
    dst = mask[1].astype(np.int64, copy=False)

    order = np.argsort(src, kind="stable")
    src_s = src[order].astype(np.int32)
    dst_s = dst[order].astype(np.int32)
    core_bounds = np.searchsorted(
        src_s, (SEG_PER_CORE * np.arange(NCORES + 1)).astype(np.int32)
    )

    percore = []
    cnt_runs = np.zeros((NCORES, NRUNS), np.int64)
    cnt_bb = np.zeros((NCORES, NRUNS, SB), np.int64)
    for k in range(NCORES):
        lo, hi = int(core_bounds[k]), int(core_bounds[k + 1])
        s = src_s[lo:hi] - SEG_PER_CORE * k
        d = dst_s[lo:hi]
        sbi = s // (P * SB)
        q = d // QROWS
        run = sbi * NQ + q
        o = np.lexsort((s, run))
        s, d, run = s[o], d[o], run[o]
        cnt_runs[k] = np.bincount(run, minlength=NRUNS)
        bb = (s - (run // NQ) * (P * SB)) // P
        cnt_bb[k] = np.bincount(run * SB + bb, minlength=NRUNS * SB).reshape(
            NRUNS, SB
        )
        percore.append((s, d, run))

    runcap = (-(-cnt_runs.max(axis=0) // P)).astype(np.int64)  # subtiles/run
    slot0 = np.zeros(NRUNS + 1, np.int64)
    slot0[1:] = np.cumsum(runcap)
    nslots = int(slot0[-1])

    # spans: per (run, bb) the union subtile range across cores
    cum = np.zeros((NCORES, NRUNS, SB + 1), np.int64)
    cum[:, :, 1:] = np.cumsum(cnt_bb, axis=2)
    span_lo = (cum[:, :, :SB] // P).min(axis=0)          # [NRUNS, SB]
    span_hi = (-(-cum[:, :, 1:] // P)).max(axis=0)       # [NRUNS, SB]
    has_edges = cnt_bb.sum(axis=0) > 0                   # [NRUNS, SB]
    span_lo = np.where(has_edges, span_lo, 0)
    span_hi = np.where(has_edges, span_lo, 0) + np.where(
        has_edges, span_hi - span_lo, 0
    )
    spancap = int((span_hi - span_lo).max())

    # gather calls: per run, chunks of <= CALL_CAP subtiles (ring limit)
    calls = []  # (run, q, chunk_sub0, nsub)
    for r in range(NRUNS):
        q = r % NQ
        for c0 in range(0, int(runcap[r]), CALL_CAP):
            nsub = min(CALL_CAP, int(runcap[r]) - c0)
            calls.append((r, q, c0, nsub))
    ncalls = len(calls)

    # iota table: iota_all[p, bb*128*spancap + c*spancap + t] = 128*bb + c
    iota_row = np.repeat(np.arange(SB * P, dtype=np.float16), spancap)
    iota_all = np.broadcast_to(iota_row[None, :], (P, SB * P * spancap)).copy()

    in_maps = []
    for k in range(NCORES):
        s, d, run = percore[k]
        cum_r = np.zeros(NRUNS, np.int64)
        cum_r[1:NRUNS] = np.cumsum(cnt_runs[k])[:-1]
        rank = np.arange(len(s), dtype=np.int64) - cum_r[run]
        pos = slot0[run] * P + rank

        # pads: srcloc sentinel (one-hot column all zero), dst idx 0
        # (gathers a real finite row; its contribution is zeroed by the
        # sentinel one-hot)
        srcloc = np.full(nslots * P, SENT, np.float16)
        srcloc[pos] = (s - (run // NQ) * (P * SB)).astype(np.float16)
        dloc = np.zeros(nslots * P, np.int16)
        dloc[pos] = (d % QROWS).astype(np.int16)

        srcloc_t = np.ascontiguousarray(srcloc.reshape(nslots, P).T)
        idx_blk = np.ascontiguousarray(dloc.reshape(nslots * 8, 16).T)
        idx16 = np.tile(idx_blk, (8, 1))
        in_maps.append(
            {
                "emb": emb16,
                "srcloc": srcloc_t,
                "dstidx": idx16,
                "iota_all": iota_all,
            }
        )

    layout = {
        "runcap": runcap,
        "slot0": slot0,
        "nslots": nslots,
        "span_lo": span_lo,
        "span_hi": span_hi,
        "spancap": spancap,
        "calls": calls,
        "ncalls": ncalls,
    }
    return in_maps, layout


def build_program(layout, repeats=1, hw_loop=False, gt_bufs=2, oh_bufs=2):
    import concourse.bass as bass
    import concourse.tile as tile
    from concourse import bacc, mybir

    slot0 = layout["slot0"]
    nslots = layout["nslots"]
    span_lo = layout["span_lo"]
    span_hi = layout["span_hi"]
    spancap = layout["spancap"]
    calls = layout["calls"]

    nc = bacc.Bacc(
        "TRN2", target_bir_lowering=False, debug=False,
        num_swdge_queues=NQUEUES, dynamic_dma_scratch_size=DMA_SCRATCH,
    )
    emb_t = nc.dram_tensor("emb", [N_SPOT, D], mybir.dt.float16,
                           kind="ExternalInput")
    srcloc_t = nc.dram_tensor("srcloc", [P, nslots], mybir.dt.float16,
                              kind="ExternalInput")
    dstidx_t = nc.dram_tensor("dstidx", [P, nslots * 8], mybir.dt.int16,
                              kind="ExternalInput")
    iota_t = nc.dram_tensor("iota_all", [P, SB * P * spancap],
                            mybir.dt.float16, kind="ExternalInput")
    out_t = nc.dram_tensor("out", [NBLK * P, D], mybir.dt.float16,
                           kind="ExternalOutput")

    sbs = []
    for sbi in range(NSB):
        nb = min(SB, NBLK - sbi * SB)
        r0 = sbi * NQ
        sb_slot0 = int(slot0[r0])
        ns_sb = int(slot0[r0 + NQ] - slot0[r0])
        regions = []
        oh_off = 0
        for bb in range(nb):
            for q in range(NQ):
                r = r0 + q
                lo = int(span_lo[r, bb])
                hi = int(span_hi[r, bb])
                w = hi - lo
                if w <= 0:
                    continue
                loc = int(slot0[r] - slot0[r0]) + lo
                regions.append((bb, q, loc, w, oh_off))
                oh_off += P * w
        cl = [ci for ci, (r, q, c0, nsub) in enumerate(calls)
              if r0 <= r < r0 + NQ]
        sbs.append((sbi, nb, sb_slot0, ns_sb, regions, oh_off, cl))
    max_ns_sb = max(s[3] for s in sbs)
    max_oh = max(s[5] for s in sbs)

    with tile.TileContext(nc) as tc, ExitStack() as ctx:
        consts = ctx.enter_context(tc.tile_pool(name="consts", bufs=1))
        gpool = ctx.enter_context(tc.tile_pool(name="gather", bufs=gt_bufs))
        ohpool = ctx.enter_context(tc.tile_pool(name="onehot", bufs=oh_bufs))
        spool = ctx.enter_context(tc.tile_pool(name="scratch", bufs=4))
        opool = ctx.enter_context(tc.tile_pool(name="outs", bufs=2))
        ppool = ctx.enter_context(
            tc.tile_pool(name="psum", bufs=8, space="PSUM")
        )

        srcloc_sb = consts.tile([P, nslots], mybir.dt.float16)
        dstidx_sb = consts.tile([P, nslots * 8], mybir.dt.int16)
        iota_sb = consts.tile([P, SB * P * spancap], mybir.dt.float16)

        out_ap = out_t.ap()
        emb_ap = emb_t.ap()
        dstidx_ap = dstidx_t.ap()

        def body():
            nc.sync.dma_start(srcloc_sb[:], srcloc_t.ap())
            nc.sync.dma_start(iota_sb[:], iota_t.ap())
            callno = 0
            for sbi, nb, sb_slot0, ns_sb, regions, oh_cols, cl in sbs:
                r0 = sbi * NQ
                nc.sync.dma_start(
                    dstidx_sb[:, sb_slot0 * 8 : (sb_slot0 + ns_sb) * 8],
                    dstidx_ap[:, sb_slot0 * 8 : (sb_slot0 + ns_sb) * 8],
                )
                gt = gpool.tile([P, max_ns_sb * D], mybir.dt.float16,
                                tag="gt")
                for ci in cl:
                    r, q, c0, nsub = calls[ci]
                    u0 = int(slot0[r] - slot0[r0]) + c0
                    s0 = int(slot0[r]) + c0
                    nc.gpsimd.dma_gather(
                        out_ap=gt[:, u0 * D : (u0 + nsub) * D].rearrange(
                            "p (c e) -> p c e", e=D
                        ),
                        in_ap=emb_ap[q * QROWS : (q + 1) * QROWS, :],
                        idxs_ap=dstidx_sb[:, s0 * 8 : (s0 + nsub) * 8],
                        num_idxs=nsub * P,
                        num_idxs_reg=nsub * P,
                        elem_size=D,
                        single_packet=False,
                        queue_num=callno % NQUEUES,
                    )
                    callno += 1
                oh = ohpool.tile([P, max_oh], mybir.dt.float16, tag="oh")
                for bb, q, loc, w, ooff in regions:
                    # oh[p, ooff + c*w + t] =
                    #   (iota_all[p, bb*128*spancap + c*spancap + t]
                    #    == srcloc[p, sb_slot0 + loc + t])
                    iot = iota_sb[:, bb * P * spancap :]
                    iot_b = bass.AP(
                        iot.tensor, iot.offset,
                        [iot.ap[0], [spancap, P], [1, w]],
                    )
                    srl = srcloc_sb[:, sb_slot0 + loc :]
                    srl_b = bass.AP(
                        srl.tensor, srl.offset, [srl.ap[0], [0, P], [1, w]]
                    )
                    oh_o = oh[:, ooff:]
                    oh_b = bass.AP(
                        oh_o.tensor, oh_o.offset,
                        [oh_o.ap[0], [w, P], [1, w]],
                    )
                    nc.vector.tensor_tensor(
                        out=oh_b, in0=iot_b, in1=srl_b,
                        op=mybir.AluOpType.is_equal,
                    )
                ot = opool.tile([P, SB * D], mybir.dt.float16, tag="ot")
                for bb in range(nb):
                    regs = [g for g in regions if g[0] == bb]
                    if not regs:
                        nc.vector.memset(ot[:, bb * D : (bb + 1) * D], 0.0)
                        continue
                    ps = ppool.tile([P, D], mybir.dt.float32, space="PSUM")
                    n_mm = sum(g[3] for g in regs)
                    i = 0
                    for _, q, loc, w, ooff in regs:
                        for t in range(w):
                            lsrc = oh[:, ooff + t : ooff + t + (P - 1) * w + 1]
                            lhsT = bass.AP(
                                lsrc.tensor, lsrc.offset,
                                [lsrc.ap[0], [w, P]],
                            )
                            nc.tensor.matmul(
                                ps[:], lhsT=lhsT,
                                rhs=gt[:, (loc + t) * D : (loc + t + 1) * D],
                                start=(i == 0), stop=(i == n_mm - 1),
                            )
                            i += 1
                    sq = spool.tile([P, D], mybir.dt.float32)
                    ss = spool.tile([P, 1], mybir.dt.float32)
                    nc.scalar.activation(
                        sq[:], ps[:], mybir.ActivationFunctionType.Square,
                        accum_out=ss[:],
                    )
                    nrm = spool.tile([P, 1], mybir.dt.float32)
                    nc.scalar.activation(
                        nrm[:], ss[:], mybir.ActivationFunctionType.Sqrt
                    )
                    nc.vector.tensor_scalar(
                        out=nrm[:], in0=nrm[:], scalar1=1e-12, scalar2=None,
                        op0=mybir.AluOpType.max,
                    )
                    nc.vector.reciprocal(nrm[:], nrm[:])
                    nc.scalar.activation(
                        ot[:, bb * D : (bb + 1) * D], ps[:],
                        mybir.ActivationFunctionType.Copy, scale=nrm[:],
                    )
                nc.sync.dma_start(
                    out_ap[sbi * SB * P : sbi * SB * P + nb * P, :].rearrange(
                        "(j p) e -> p j e", p=P
                    ),
                    ot[:, : nb * D].rearrange("p (j e) -> p j e", e=D),
                )

        if hw_loop and repeats > 1:
            with tc.For_i(0, repeats):
                body()
        else:
            for _ in range(repeats):
                body()

    nc.compile()
    return nc


_PROGRAM_CACHE = {}


def _get_program(layout):
    key = (
        layout["runcap"].tobytes(),
        layout["span_lo"].tobytes(),
        layout["span_hi"].tobytes(),
    )
    if key not in _PROGRAM_CACHE:
        _PROGRAM_CACHE[key] = build_program(layout)
    return _PROGRAM_CACHE[key]


def kernel(**inputs):
    emb = inputs["emb"]
    mask = inputs["mask"]
    in_maps, layout = preprocess(emb, mask)
    nc = _get_program(layout)

    import time
    from concourse.bass_utils import run_bass_kernel_spmd

    res = None
    err = None
    for attempt in range(3):
        try:
            res = run_bass_kernel_spmd(nc, in_maps, core_ids=list(range(NCORES)))
            break
        except Exception as e:  # noqa: BLE001 - transient axon UNAVAILABLE
            err = e
            time.sleep(3)
    if res is None:
        raise err
    out = np.empty((N_SPOT, D), np.float32)
    for k in range(NCORES):
        out[k * SEG_PER_CORE : (k + 1) * SEG_PER_CORE] = res.results[k]["out"][
            :SEG_PER_CORE
        ]
    return out
